# revision 58
# baseline (speedup 1.0000x reference)
"""Trainium2 Bass kernel for nn_ModalityAdaptiveModule.

Reference computation (B=2, S=4096, D=512):
    tn = LN(text, g_t, b_t); im = LN(img, g_i, b_i)
    norms = concat([tn, im])                  # [4, S, D]
    K = concat([tn@Wkt.T+bkt, im@Wki.T+bki])  # [4, S, D]
    V = concat([tn@Wvt.T+bvt, im@Wvi.T+bvi])
    q = norms@Wq.T + bq
    attn = softmax(q@K.T / sqrt(D)); x = attn@V; x = x@Wo.T + bo
    out = concat([LN(x, g_t, b_t), LN(x, g_i, b_i)])  # [8, S, D]

Sharding: 8 cores = (attention batch b in 0..3) x (query half h in 0..1).
Each core gets its batch's full [S, D] input with its own query half
permuted to the front (key order is irrelevant to attention), computes
K/V over all S, Q over its 2048 rows, and writes both final-LN outputs
for its rows.

Two device variants:

LEAN (used when all bias vectors are zero and the final LN gains are
ones -- which setup_inputs() guarantees): everything flows in bf16,
  - x arrives bf16; LN stats via bn_stats on the natural layout; the
    LN apply is a single DVE tensor_scalar ((x + -mu) * rsig); the
    normalized A (bf16) is transposed by the DMA xbar
    (dma_start_transpose), so the per-chunk PE transposes and their
    PSUM evacuation copies disappear (chunk 0 still uses a PE
    transpose: shorter latency than the xbar DMA pipeline at start).
  - no bias adds anywhere (all are zero).
  - softmax denominator is skipped entirely: with bo == 0 the final
    LN is invariant to per-token scaling, so LN(x_unnorm @ Wo) equals
    LN((x_unnorm/sums) @ Wo) exactly; no sum accumulation, no
    ones-matmul, no reciprocal.
  - the output projection is folded into the V projection on the host
    ((U V) Wo = U (V Wo), V Wo = A (Wo Wv)^T), and the attention
    matmul uses U as the stationary operand so y comes out token-major
    -- the entire O-projection matmul stage disappears.
  - final out_t == out_i (gains ones, biases zero), so the final
    normalized tile is DMA'd to both output slots.
  - PSUM evacuations: K/Q on ACT, V on DVE (GPSIMD cannot touch PSUM).
  - a short warmup matmul chain starts the PE p-state ramp at t~0.
  - pool depths / block taper in `cfg` are tuned against the
    TimelineSim cost model (the grading metric).

GENERAL (any other inputs): the original fully-general pipeline
(f32r matmuls, on-chip PE transposes, bias adds, per-modality final
affine, softmax sums via DVE accumulation + ones-matmul + diagonal
extraction).  Slower but correct for arbitrary inputs.
"""

import numpy as np
import ml_dtypes

import concourse.bass as bass
import concourse.mybir as mybir
import concourse.tile as tile
from concourse import bacc
from concourse.bass_utils import run_bass_kernel_spmd

AF = mybir.ActivationFunctionType
OP = mybir.AluOpType

# Pin ALL activations to the one table set that contains every function this
# kernel uses (exp, ln, copy, identity). The default chooser maps exp and ln
# to different sets, inserting a ~1.3us LoadActFuncSet per alternation
# (~120us/kernel). Emptying the other sets (order/ids preserved) forces a
# single load.
import concourse.hw_specs as _hw_specs
import functools as _functools

_ORIG_GET_ACT_TABLES = _hw_specs.get_activation_tables


@_functools.cache
def _pinned_act_tables(module_arch):
    full = _ORIG_GET_ACT_TABLES(module_arch)
    keep = "natural_log_exp_and_others"
    return {name: (funcs if name == keep else set())
            for name, funcs in full.items()}


ENABLE_ACT_PIN = True
if ENABLE_ACT_PIN:
    _hw_specs.get_activation_tables = _pinned_act_tables
    bacc.get_activation_tables = _pinned_act_tables
F32 = mybir.dt.float32
F32R = mybir.dt.float32r
BF16 = mybir.dt.bfloat16

D = 512
S = 4096          # keys per batch
TQ = 2048         # queries per core
DT = 4            # d tiles of 128
NKT = S // 128    # 32 key tiles
TC = 256          # phase-1 token chunk
NCH = S // TC     # 16 chunks
TQB = 512         # tq block
EPS = 1e-5


def build_kernel_lean(cfg=None):
    base = dict(p1x=5, p1a=6, p1t=3, p1s=6, p1pk=2, p1pq=2, p1pv=1, nsca=2,
                nwarm=24, nwarm2=20, vevac="dve", xbar_eng="scalar",
                fill={}, taper=(256, 256), opk=26, p2u=18, fs=2, splitn2=False, fineng=False, ptc=3, fusedma=False, p2s=2, p2n=2,
                p2st=3, psc=2, psy=2, aina=False)
    base.update(cfg or {})
    cfg = base
    nc = bacc.Bacc("TRN2", target_bir_lowering=False, debug=False,
                   enable_asserts=True, num_devices=8)

    x_d = nc.dram_tensor("x", [S, D], BF16, kind="ExternalInput").ap()
    gm_d = nc.dram_tensor("gm", [D, D], BF16, kind="ExternalInput").ap()
    wvo_d = nc.dram_tensor("wvo", [D, D], BF16, kind="ExternalInput").ap()
    identb_d = nc.dram_tensor("identb", [128, 128], BF16,
                              kind="ExternalInput").ap()
    out_d = nc.dram_tensor("out2", [2, TQ, D], F32, kind="ExternalOutput").ap()

    with tile.TileContext(nc) as tc:
        with (
            tc.tile_pool(name="persist", bufs=1) as persist,
            tc.tile_pool(name="resident", bufs=1) as resident,
        ):
            # ---- critical-path first: warmup operand + first x chunk ----
            warm = persist.tile([128, 128], BF16)
            nc.vector.memset(warm[:], 0.25)
            xc0 = persist.tile([128, 2, D], BF16)
            nc.sync.dma_start(
                xc0[:], x_d[0:TC, :].rearrange("(s p) d -> p s d", p=128))
            identb = persist.tile([128, 128], BF16)
            nc.sync.dma_start(identb[:], identb_d)
            eps_t = persist.tile([128, 1], F32)
            nc.vector.memset(eps_t[:], EPS)

            # ---- resident big tensors ----
            # scores contract directly against AT (K-proj folded into the
            # query-side M = gq gk^T); attn contracts against Anat (V-proj
            # folded into the output-side Wvo = Wo Wv)
            AT = resident.tile([128, DT, S], BF16)
            QT = resident.tile([128, DT, TQ], BF16)
            Anat = resident.tile([128, NKT, D], BF16)
            wvo_s = resident.tile([128, DT, D], BF16)

            def rsig_lnexp(pool, var_ap, tag):
                """1/sqrt(var+eps) via exp(-0.5*ln(var+eps)) -- stays in the
                exp/ln ACT table set (no LoadActFuncSet thrash)."""
                lnv = pool.tile([128, 1], F32, tag=f"lnv{tag}", name=f"lnv{tag}")
                nc.scalar.activation(lnv[:], var_ap, AF.Ln, bias=eps_t[:, 0:1],
                                     scale=1.0)
                rs = pool.tile([128, 1], F32, tag=f"rsx{tag}", name=f"rsx{tag}")
                nc.scalar.activation(rs[:], lnv[:], AF.Exp, scale=-0.5)
                return rs

            # ====== PHASE 1 (region A): chunks 0-7, LN + xbar + K/V/Q ======
            # Region B overlaps chunks 8-15 (K/V only; Q is complete) with
            # block-0 attention k-iters 0..15 (which need only KT/Vb tiles
            # from chunks 0-7 and QT queries 0-511), so phase-1 supply
            # stalls are filled with attention matmuls and the PE p-state
            # ramp never resets.  PSUM: A = pk2+pq2+pv1+tp1+psc2, B = psc2+
            # pxv4+pk1+pv1, C = psc2+pxv4 (<= 8 banks each).
            with (
                tc.tile_pool(name="p1w", bufs=1) as p1w,
                tc.tile_pool(name="p1x", bufs=cfg["p1x"]) as p1x,
                tc.tile_pool(name="p1a", bufs=cfg["p1a"]) as p1a,
                tc.tile_pool(name="p1t", bufs=cfg["p1t"]) as p1t,
                tc.tile_pool(name="p1s", bufs=cfg["p1s"]) as p1s,
                tc.tile_pool(name="p2u", bufs=cfg["p2u"]) as p2u,
                tc.tile_pool(name="p2s", bufs=cfg["p2s"]) as p2s,
                tc.tile_pool(name="p2st", bufs=cfg["p2st"]) as p2st,
                tc.tile_pool(name="p2n", bufs=cfg["p2n"]) as p2n,
            ):
                xcs = {0: xc0}

                def load_chunk(c):
                    if c >= NCH or c in xcs:
                        return
                    xc = p1x.tile([128, 2, D], BF16, tag="xc", name=f"xc{c}")
                    nc.sync.dma_start(
                        xc[:], x_d[c * TC:(c + 1) * TC, :].rearrange(
                            "(s p) d -> p s d", p=128))
                    xcs[c] = xc

                def ln_chunk(c, tp_pool=None):
                    xc = xcs.pop(c)
                    Ac = Anat[:, 2 * c:2 * c + 2, :]
                    for s in range(2):
                        # DVE chain is self-contained (stats -> aggr -> -mu)
                        # so the scheduler can't head-of-line block the apply;
                        # the apply is one DVE tensor_scalar (x + -mu) * rsig.
                        stats = p1s.tile([128, 6], F32, tag="st", name=f"st{c}_{s}")
                        nc.vector.bn_stats(stats[:], xc[:, s, :])
                        mv = p1s.tile([128, 2], F32, tag="mv", name=f"mv{c}_{s}")
                        nc.vector.bn_aggr(mv[:], stats[:])
                        negmu = p1s.tile([128, 1], F32, tag="ngm", name=f"ngm{c}_{s}")
                        nc.vector.tensor_scalar(
                            out=negmu[:], in0=mv[:, 0:1], scalar1=-1.0,
                            scalar2=None, op0=OP.mult, op1=OP.bypass)
                        rs = rsig_lnexp(p1s, mv[:, 1:2], "1")
                        nc.vector.tensor_scalar(
                            out=Ac[:, s, :], in0=xc[:, s, :],
                            scalar1=negmu[:, 0:1], scalar2=rs[:, 0:1],
                            op0=OP.add, op1=OP.mult)
                    AcT8 = AT[:, :, 0:1]  # placeholder, written below
                    if tp_pool is not None:
                        # first chunks: PE transpose + DVE evac (short
                        # latency) beats the xbar pipeline at kernel start
                        for j in range(8):
                            tp = tp_pool.tile([128, 128], BF16, tag="tp",
                                              name=f"tp{c}_{j}")
                            s, dt = j // DT, j % DT
                            nc.tensor.transpose(
                                tp[:], Ac[:, s, dt * 128:(dt + 1) * 128],
                                identb[:])
                            nc.vector.tensor_copy(
                                AT[:, dt, (2 * c + s) * 128:
                                   (2 * c + s + 1) * 128], tp[:])
                    else:
                        # xbar into a staging tile (contiguous dest required
                        # by the hardware), then strided SBUF copy into AT
                        AcT8 = p1t.tile([128, 8, 128], BF16, tag="act",
                                        name=f"act{c}")
                        getattr(nc, cfg["xbar_eng"]).dma_start_transpose(
                            AcT8[:], Ac[:])
                        for s in range(2):
                            nc.vector.tensor_copy(
                                AT[:, :, (2 * c + s) * 128:
                                   (2 * c + s + 1) * 128],
                                AcT8[:, s * DT:(s + 1) * DT, :])
                    return None

                def proj_chunk(c, _unused, qpool):
                    # only the query-side projection remains: QM = A @ M
                    # (M = gq gk^T folds the K projection into the query)
                    for s in range(2):
                        kt = c * 2 + s
                        if kt < TQ // 128:
                            pq = qpool.tile([128, DT, 128], F32, tag="pq",
                                            name=f"pq{kt}")
                            for o in range(DT):
                                for i in range(DT):
                                    nc.tensor.matmul(
                                        pq[:, o, :],
                                        gm_s[:, i, o * 128:(o + 1) * 128],
                                        AT[:, i, kt * 128:(kt + 1) * 128],
                                        start=(i == 0), stop=(i == DT - 1))
                            nc.scalar.copy(
                                QT[:, :, kt * 128:(kt + 1) * 128], pq[:])

                # -------- region A: chunks 0-7 --------
                _psc_cm = tc.tile_pool(name="psc", bufs=cfg["psc"],
                                       space="PSUM")
                psc = _psc_cm.__enter__()
                BLOCKS = [(0, 512), (512, 512), (1024, 512)]
                q0t = 1536
                for w in cfg["taper"]:
                    BLOCKS.append((q0t, w))
                    q0t += w
                assert q0t == TQ
                q0b, w0 = BLOCKS[0]
                Us0 = [None] * NKT

                def score_step(k):
                    # block-0 scores/exp only: needs QT queries 0-511
                    # (chunks 0-1) and KT tile k (chunk k//2); fills
                    # phase-1 PE stalls, attnV catches up in region B
                    ps = psc.tile([128, w0], F32, tag="ps", name=f"ps0_{k}")
                    for i in range(DT):
                        nc.tensor.matmul(
                            ps[:], AT[:, i, k * 128:(k + 1) * 128],
                            QT[:, i, q0b:q0b + w0],
                            start=(i == 0), stop=(i == DT - 1))
                    U = p2u.tile([128, w0], BF16, tag="ut", name=f"ut0_{k}")
                    nc.scalar.activation(U[:], ps[:], AF.Exp)
                    Us0[k] = U

                with (
                    tc.tile_pool(name="p1pk", bufs=cfg["p1pk"], space="PSUM") as p1pk,
                    tc.tile_pool(name="p1pq", bufs=cfg["p1pq"], space="PSUM") as p1pq,
                    tc.tile_pool(name="p1tp", bufs=1, space="PSUM") as p1tp,
                ):
                    # PE p-state warmup: cheap matmuls keep PE busy from t~0
                    NWARM = cfg["nwarm"]
                    wps = p1pk.tile([128, 128], F32, tag="pk", name="warm")
                    for i in range(NWARM):
                        nc.tensor.matmul(wps[:], warm[:], warm[:],
                                         start=(i == 0), stop=(i == NWARM - 1))

                    # weight loads interleaved with early x chunks so the
                    # DMA FIFO serves the first-chunk critical path promptly
                    gm_s = p1w.tile([128, DT, D], BF16)
                    nc.sync.dma_start(
                        gm_s[:], gm_d.rearrange("(i p) o -> p i o", p=128))
                    load_chunk(1)
                    nc.sync.dma_start(
                        wvo_s[:], wvo_d.rearrange("(i p) o -> p i o", p=128))

                    ln_chunk(0, tp_pool=p1tp)
                    ptc = cfg["ptc"]
                    # second warmup batch runs while DVE evacuates the
                    # chunk-0 transposes
                    if cfg["nwarm2"]:
                        wps2 = p1pk.tile([128, 128], F32, tag="pk",
                                         name="warm2")
                        for i in range(cfg["nwarm2"]):
                            nc.tensor.matmul(wps2[:], warm[:], warm[:],
                                             start=(i == 0),
                                             stop=(i == cfg["nwarm2"] - 1))
                    nsca, fs = cfg["nsca"], cfg["fs"]
                    a_scores = 0
                    a_attn = 0

                    def attn_step(kk):
                        raise RuntimeError("attn fills disabled in region A")
                    for c in range(1, NCH // 2 + 1):
                        load_chunk(c + 1)
                        ln_chunk(c, tp_pool=(p1tp if c < ptc else None))
                        proj_chunk(c - 1, None, p1pq)
                        if c >= fs:
                            # scores k needs KT tile k (chunk k//2) and QT
                            # queries 0-511 (chunks 0-1); emit contiguously
                            budget = nsca * (c - fs + 1)
                            while (a_scores < budget
                                   and a_scores < NKT // 2
                                   and a_scores <= 2 * c - 3):
                                score_step(a_scores)
                                a_scores += 1
                                while (cfg["aina"]
                                       and a_attn < a_scores - 2):
                                    attn_step(a_attn)
                                    a_attn += 1

                with (
                    tc.tile_pool(name="pxv", bufs=1, space="PSUM") as pxv,
                ):
                    def k_step(blk, q0, w, pxs, Us, k, mode="both"):
                        # skewed: scores/exp for k, attnV for k-1
                        if k < NKT and mode in ("both", "score"):
                            ps = psc.tile([128, w], F32, tag="ps",
                                          name=f"ps{blk}_{k}")
                            for i in range(DT):
                                nc.tensor.matmul(
                                    ps[:], AT[:, i, k * 128:(k + 1) * 128],
                                    QT[:, i, q0:q0 + w],
                                    start=(i == 0), stop=(i == DT - 1))
                            U = p2u.tile([128, w], BF16, tag="ut",
                                         name=f"ut{blk}_{k}")
                            nc.scalar.activation(U[:], ps[:], AF.Exp)
                            Us[k] = U
                        if k >= 1 and mode in ("both", "attn"):
                            kk = k - 1
                            Ukk = Us[kk]
                            # Z_T[d, q] = sum_k A[k, d] U[k, q]: the LN'd
                            # input itself is the folded V
                            for dt in range(DT):
                                nc.tensor.matmul(
                                    pxs[dt][:],
                                    Anat[:, kk, dt * 128:(dt + 1) * 128],
                                    Ukk[:], start=(kk == 0),
                                    stop=(kk == NKT - 1))
                            Us[kk] = None

                    # ---- region B: chunks 8-15 LN + block-0 catch-up ----
                    pxs0 = [pxv.tile([128, w0], F32, tag=f"px{dt}",
                                     name=f"px{dt}_0") for dt in range(DT)]
                    next_s = a_scores   # next score k to emit
                    next_a = a_attn     # next attnV kk to emit
                    if True:
                        for c in range(NCH // 2, NCH):
                            load_chunk(c + 2)
                            if c + 1 < NCH:
                                ln_chunk(c + 1)
                            # pace so both scores and attnV finish k<16 by
                            # region-B end; attnV kk stays behind scores
                            rem = NCH - c
                            while (16 - next_s) > 0 and \
                                  (16 - next_s) >= 2 * (rem - 1):
                                score_step(next_s)
                                next_s += 1
                            while (16 - next_a) > 0 and \
                                  (16 - next_a) >= 2 * (rem - 1) and \
                                  next_a < next_s - 1:
                                k_step(0, q0b, w0, pxs0, Us0, next_a + 1,
                                       mode="attn")
                                next_a += 1
                        while next_s < 16:
                            score_step(next_s)
                            next_s += 1
                        while next_a < 15:
                            k_step(0, q0b, w0, pxs0, Us0, next_a + 1,
                                   mode="attn")
                            next_a += 1

                    # -------- region C: rest of block 0 + blocks 1+ ----------
                    with tc.tile_pool(name="psum_y", bufs=cfg["psy"],
                                      space="PSUM") as psum_y:
                        def oproj_ln(q0, w, zT):
                            for j in range(w // 128):
                                py = psum_y.tile([128, D], F32, tag="py",
                                                 name=f"py{q0}_{j}")
                                for dt in range(DT):
                                    nc.tensor.matmul(
                                        py[:], zT[:, dt, j * 128:(j + 1) * 128],
                                        wvo_s[:, dt, :], start=(dt == 0),
                                        stop=(dt == DT - 1))
                                y = py
                                stats = p2st.tile([128, 6], F32, tag="st2",
                                                  name=f"st2_{q0}_{j}")
                                nc.vector.bn_stats(stats[:], y[:])
                                mv = p2st.tile([128, 2], F32, tag="mv2",
                                               name=f"mv2_{q0}_{j}")
                                nc.vector.bn_aggr(mv[:], stats[:])
                                rs2 = rsig_lnexp(p2st, mv[:, 1:2], "2")
                                nmr2 = p2st.tile([128, 1], F32, tag="nmr2",
                                                 name=f"nmr2_{q0}_{j}")
                                nc.vector.tensor_scalar(
                                    out=nmr2[:], in0=mv[:, 0:1],
                                    scalar1=rs2[:, 0:1], scalar2=-1.0,
                                    op0=OP.mult, op1=OP.mult)
                                n2 = p2n.tile([128, D], F32, tag="n2",
                                              name=f"n2_{q0}_{j}")
                                nc.scalar.activation(n2[:], y[:], AF.Identity,
                                                     bias=nmr2[:, 0:1],
                                                     scale=rs2[:, 0:1])
                                r0 = q0 + j * 128
                                nc.sync.dma_start(
                                    out_d[0, r0:r0 + 128, :], n2[:])
                                nc.sync.dma_start(
                                    out_d[1, r0:r0 + 128, :], n2[:])

                        def evac_block(blk, q0, w, pxs):
                            # evacuate un-normalized Z_T (sums not needed)
                            zT = p2s.tile([128, DT, w], BF16, tag="xt",
                                          name=f"zT{blk}")
                            for dt in range(DT):
                                nc.scalar.copy(zT[:, dt, :], pxs[dt][:])
                            return zT

                        for k in range(NKT // 2, NKT + 1):
                            k_step(0, q0b, w0, pxs0, Us0, k)
                        prev_oproj = (q0b, w0, evac_block(0, q0b, w0, pxs0))

                        for blk, (q0, w) in enumerate(BLOCKS):
                            if blk == 0:
                                continue
                            pxs = [pxv.tile([128, w], F32, tag=f"px{dt}",
                                            name=f"px{dt}_{blk}")
                                   for dt in range(DT)]
                            Us = [None] * NKT
                            for k in range(NKT + 1):
                                if k == cfg["opk"] and prev_oproj is not None:
                                    oproj_ln(*prev_oproj)
                                    prev_oproj = None
                                k_step(blk, q0, w, pxs, Us, k)
                            prev_oproj = (q0, w, evac_block(blk, q0, w, pxs))
                        oproj_ln(*prev_oproj)
                _psc_cm.__exit__(None, None, None)
    nc.compile()
    return nc


def build_kernel_general():
    nc = bacc.Bacc("TRN2", target_bir_lowering=False, debug=False,
                   enable_asserts=True, num_devices=8)

    x_d = nc.dram_tensor("x", [S, D], F32, kind="ExternalInput").ap()
    gqt_d = nc.dram_tensor("gqt", [D, D], F32R, kind="ExternalInput").ap()
    gkt_d = nc.dram_tensor("gkt", [D, D], F32R, kind="ExternalInput").ap()
    gvt_d = nc.dram_tensor("gvt", [D, D], F32R, kind="ExternalInput").ap()
    wot_d = nc.dram_tensor("wot", [D, D], F32R, kind="ExternalInput").ap()
    cq_d = nc.dram_tensor("cq", [D], F32, kind="ExternalInput").ap()
    ck_d = nc.dram_tensor("ck", [D], F32, kind="ExternalInput").ap()
    cv_d = nc.dram_tensor("cv", [D], F32, kind="ExternalInput").ap()
    bo_d = nc.dram_tensor("bo", [D], F32, kind="ExternalInput").ap()
    g2t_d = nc.dram_tensor("g2t", [D], F32, kind="ExternalInput").ap()
    b2t_d = nc.dram_tensor("b2t", [D], F32, kind="ExternalInput").ap()
    g2i_d = nc.dram_tensor("g2i", [D], F32, kind="ExternalInput").ap()
    b2i_d = nc.dram_tensor("b2i", [D], F32, kind="ExternalInput").ap()
    ident_d = nc.dram_tensor("ident", [128, 128], F32, kind="ExternalInput").ap()
    onesr_d = nc.dram_tensor("onesr", [128, 128], F32R, kind="ExternalInput").ap()
    out_d = nc.dram_tensor("out2", [2, TQ, D], F32, kind="ExternalOutput").ap()

    def bcast(vec_ap, parts=128):
        return bass.AP(tensor=vec_ap.tensor, offset=vec_ap.offset,
                       ap=[[0, parts]] + list(vec_ap.ap))

    with tile.TileContext(nc) as tc:
        with (
            tc.tile_pool(name="persist", bufs=1) as persist,
            tc.tile_pool(name="resident", bufs=1) as resident,
        ):
            # ---- critical-path first: identity + first x chunk DMA ----
            ident = persist.tile([128, 128], F32)
            nc.sync.dma_start(ident[:], ident_d)
            xc0 = persist.tile([128, 2, D], F32)
            nc.sync.dma_start(
                xc0[:], x_d[0:TC, :].rearrange("(s p) d -> p s d", p=128))
            eps_t = persist.tile([128, 1], F32)
            nc.vector.memset(eps_t[:], EPS)
            ones_bf = persist.tile([128, 128], BF16)
            nc.vector.memset(ones_bf[:], 1.0)
            ones_r = persist.tile([128, 128], F32R)
            nc.sync.dma_start(ones_r[:], onesr_d)
            cq_s = persist.tile([128, DT], F32)
            nc.sync.dma_start(cq_s[:], cq_d.rearrange("(o p) -> p o", p=128))
            ck_s = persist.tile([128, DT], F32)
            nc.sync.dma_start(ck_s[:], ck_d.rearrange("(o p) -> p o", p=128))
            cv_rep = persist.tile([128, D], F32)
            nc.gpsimd.dma_start(cv_rep[:], bcast(cv_d))

            # ---- resident big tensors ----
            KT = resident.tile([128, DT, S], F32R)
            QT = resident.tile([128, DT, TQ], F32R)
            Vb = resident.tile([128, NKT, D], BF16)
            wot_s = resident.tile([128, DT, D], F32R)

            def rsig_lnexp(pool, var_ap, tag):
                lnv = pool.tile([128, 1], F32, tag=f"lnv{tag}", name=f"lnv{tag}")
                nc.scalar.activation(lnv[:], var_ap, AF.Ln, bias=eps_t[:, 0:1],
                                     scale=1.0)
                rs = pool.tile([128, 1], F32, tag=f"rsx{tag}", name=f"rsx{tag}")
                nc.scalar.activation(rs[:], lnv[:], AF.Exp, scale=-0.5)
                return rs

            # ================= PHASE 1: LN + transpose + QKV =================
            with (
                tc.tile_pool(name="p1w", bufs=1) as p1w,
                tc.tile_pool(name="p1x", bufs=2) as p1x,
                tc.tile_pool(name="p1s", bufs=3) as p1s,
                tc.tile_pool(name="p1ps", bufs=2, space="PSUM") as p1ps,
                tc.tile_pool(name="p1pk", bufs=3, space="PSUM") as p1pk,
                tc.tile_pool(name="p1pv", bufs=2, space="PSUM") as p1pv,
            ):
                gkt_s = p1w.tile([128, DT, D], F32R)
                nc.sync.dma_start(gkt_s[:], gkt_d.rearrange("(i p) o -> p i o", p=128))
                gvt_s = p1w.tile([128, DT, D], F32R)
                nc.sync.dma_start(gvt_s[:], gvt_d.rearrange("(i p) o -> p i o", p=128))
                gqt_s = p1w.tile([128, DT, D], F32R)
                nc.sync.dma_start(gqt_s[:], gqt_d.rearrange("(i p) o -> p i o", p=128))

                def ln_transpose(c):
                    if c == 0:
                        xc = xc0
                    else:
                        xc = p1x.tile([128, 2, D], F32, tag="xc", name=f"xc{c}")
                        nc.sync.dma_start(
                            xc[:], x_d[c * TC:(c + 1) * TC, :].rearrange(
                                "(s p) d -> p s d", p=128))
                    Ac = p1x.tile([128, 2, D], F32, tag="ac", name=f"ac{c}", bufs=3)
                    AcT = p1x.tile([128, DT, TC], F32R, tag="act", name=f"act{c}", bufs=3)
                    for s in range(2):
                        stats = p1s.tile([128, 6], F32, tag="st", name=f"st{c}_{s}")
                        nc.vector.bn_stats(stats[:], xc[:, s, :])
                        mv = p1s.tile([128, 2], F32, tag="mv", name=f"mv{c}_{s}")
                        nc.vector.bn_aggr(mv[:], stats[:])
                        rs = rsig_lnexp(p1s, mv[:, 1:2], "1")
                        nmr = p1s.tile([128, 1], F32, tag="nmr", name=f"nmr{c}_{s}")
                        nc.vector.tensor_scalar(
                            out=nmr[:], in0=mv[:, 0:1], scalar1=rs[:, 0:1],
                            scalar2=-1.0, op0=OP.mult, op1=OP.mult)
                        nc.scalar.activation(Ac[:, s, :], xc[:, s, :], AF.Identity,
                                             bias=nmr[:, 0:1], scale=rs[:, 0:1])
                        for dt in range(DT):
                            tp = p1ps.tile([128, 128], F32, tag="tp",
                                           name=f"tp{c}_{s}_{dt}")
                            nc.tensor.transpose(
                                tp[:], Ac[:, s, dt * 128:(dt + 1) * 128], ident[:])
                            nc.vector.tensor_copy(
                                AcT[:, dt, s * 128:(s + 1) * 128], tp[:])
                    return AcT

                def projections(c, AcT):
                    for o in range(DT):
                        pk = p1pk.tile([128, TC], F32, tag="pk", name=f"pk{c}_{o}")
                        for i in range(DT):
                            nc.tensor.matmul(
                                pk[:], gkt_s[:, i, o * 128:(o + 1) * 128],
                                AcT[:, i, :], start=(i == 0), stop=(i == DT - 1))
                        nc.scalar.activation(KT[:, o, c * TC:(c + 1) * TC], pk[:],
                                             AF.Identity, bias=ck_s[:, o:o + 1],
                                             scale=1.0)
                    for s in range(2):
                        pv = p1pv.tile([128, D], F32, tag="pv", name=f"pv{c}_{s}")
                        for i in range(DT):
                            nc.tensor.matmul(
                                pv[:], AcT[:, i, s * 128:(s + 1) * 128],
                                gvt_s[:, i, :], start=(i == 0), stop=(i == DT - 1))
                        nc.vector.tensor_add(Vb[:, c * 2 + s, :], pv[:], cv_rep[:])
                    if c < NCH // 2:
                        for o in range(DT):
                            pq = p1pk.tile([128, TC], F32, tag="pk",
                                           name=f"pq{c}_{o}")
                            for i in range(DT):
                                nc.tensor.matmul(
                                    pq[:], gqt_s[:, i, o * 128:(o + 1) * 128],
                                    AcT[:, i, :], start=(i == 0), stop=(i == DT - 1))
                            nc.scalar.activation(QT[:, o, c * TC:(c + 1) * TC],
                                                 pq[:], AF.Identity,
                                                 bias=cq_s[:, o:o + 1], scale=1.0)

                prev = ln_transpose(0)
                for c in range(1, NCH):
                    cur = ln_transpose(c)
                    projections(c - 1, prev)
                    prev = cur
                projections(NCH - 1, prev)

            nc.sync.dma_start(wot_s[:], wot_d.rearrange("(i p) o -> p i o", p=128))

            # ============ PHASE 2/3: attention + out-proj + final LN ============
            with (
                tc.tile_pool(name="p2u", bufs=8) as p2u,
                tc.tile_pool(name="p2s", bufs=2) as p2s,
                tc.tile_pool(name="p2y", bufs=2) as p2y,
                tc.tile_pool(name="p2o", bufs=2) as p2o,
                tc.tile_pool(name="p2st", bufs=3) as p2st,
                tc.tile_pool(name="p2sum", bufs=2) as p2sum,
                tc.tile_pool(name="p2c", bufs=1) as p2c,
                tc.tile_pool(name="psc", bufs=3, space="PSUM") as psc,
                tc.tile_pool(name="pxv", bufs=1, space="PSUM") as pxv,
                tc.tile_pool(name="psum_y", bufs=1, space="PSUM") as psum_y,
            ):
                bo_rep = p2c.tile([128, D], F32)
                nc.gpsimd.dma_start(bo_rep[:], bcast(bo_d))
                g2t_rep = p2c.tile([128, D], F32)
                nc.gpsimd.dma_start(g2t_rep[:], bcast(g2t_d))
                b2t_rep = p2c.tile([128, D], F32)
                nc.gpsimd.dma_start(b2t_rep[:], bcast(b2t_d))
                g2i_rep = p2c.tile([128, D], F32)
                nc.gpsimd.dma_start(g2i_rep[:], bcast(g2i_d))
                b2i_rep = p2c.tile([128, D], F32)
                nc.gpsimd.dma_start(b2i_rep[:], bcast(b2i_d))

                def oproj_ln(q0, w, xT, rcp):
                    for j in range(w // 128):
                        py = psum_y.tile([128, D], F32, tag="py",
                                         name=f"py{q0}_{j}")
                        for dt in range(DT):
                            nc.tensor.matmul(
                                py[:], xT[:, dt, j * 128:(j + 1) * 128],
                                wot_s[:, dt, :], start=(dt == 0),
                                stop=(dt == DT - 1))
                        y = p2y.tile([128, D], F32, tag="y", name=f"y{q0}_{j}")
                        nc.vector.tensor_scalar_mul(y[:], py[:], rcp[:, j:j + 1])
                        nc.vector.tensor_add(y[:], y[:], bo_rep[:])
                        stats = p2st.tile([128, 6], F32, tag="st2",
                                          name=f"st2_{q0}_{j}")
                        nc.vector.bn_stats(stats[:], y[:])
                        mv = p2st.tile([128, 2], F32, tag="mv2",
                                       name=f"mv2_{q0}_{j}")
                        nc.vector.bn_aggr(mv[:], stats[:])
                        rs2 = rsig_lnexp(p2st, mv[:, 1:2], "2")
                        nmr2 = p2st.tile([128, 1], F32, tag="nmr2",
                                         name=f"nmr2_{q0}_{j}")
                        nc.vector.tensor_scalar(
                            out=nmr2[:], in0=mv[:, 0:1], scalar1=rs2[:, 0:1],
                            scalar2=-1.0, op0=OP.mult, op1=OP.mult)
                        n2 = p2y.tile([128, D], F32, tag="n2", name=f"n2_{q0}_{j}")
                        nc.scalar.activation(n2[:], y[:], AF.Identity,
                                             bias=nmr2[:, 0:1], scale=rs2[:, 0:1])
                        r0 = q0 + j * 128
                        for m, (g_rep, b_rep) in enumerate(
                                [(g2t_rep, b2t_rep), (g2i_rep, b2i_rep)]):
                            om = p2o.tile([128, D], F32, tag=f"om{m}",
                                          name=f"om{m}_{q0}_{j}")
                            nc.vector.tensor_mul(om[:], n2[:], g_rep[:])
                            nc.vector.tensor_add(om[:], om[:], b_rep[:])
                            nc.sync.dma_start(out_d[m, r0:r0 + 128, :], om[:])

                prev_oproj = None
                BLOCKS = [(0, 512), (512, 512), (1024, 512),
                          (1536, 256), (1792, 256)]
                for blk, (q0, w) in enumerate(BLOCKS):
                    pxs = [pxv.tile([128, w], F32, tag=f"px{dt}",
                                    name=f"px{dt}_{blk}") for dt in range(DT)]
                    psm = psc.tile([128, w], F32, tag="ps", name=f"psm{blk}")
                    sacc = p2sum.tile([128, w], F32R, tag="sacc",
                                      name=f"sacc{blk}")
                    Us = [None] * NKT
                    for k in range(NKT + 1):
                        if k == 12 and prev_oproj is not None:
                            oproj_ln(*prev_oproj)
                            prev_oproj = None
                        if k < NKT:
                            ps = psc.tile([128, w], F32, tag="ps",
                                          name=f"ps{blk}_{k}")
                            for i in range(DT):
                                nc.tensor.matmul(
                                    ps[:], KT[:, i, k * 128:(k + 1) * 128],
                                    QT[:, i, q0:q0 + w],
                                    start=(i == 0), stop=(i == DT - 1))
                            U = p2u.tile([128, w], BF16, tag="ut",
                                         name=f"ut{blk}_{k}")
                            nc.scalar.activation(U[:], ps[:], AF.Exp)
                            Us[k] = U
                        if k >= 1:
                            kk = k - 1
                            Ukk = Us[kk]
                            for dt in range(DT):
                                nc.tensor.matmul(
                                    pxs[dt][:], Vb[:, kk, dt * 128:(dt + 1) * 128],
                                    Ukk[:], start=(kk == 0), stop=(kk == NKT - 1))
                            if kk == 0:
                                nc.vector.tensor_copy(sacc[:], Ukk[:])
                            else:
                                nc.vector.tensor_add(sacc[:], sacc[:], Ukk[:])
                            Us[kk] = None
                    nc.tensor.matmul(psm[:], ones_r[:], sacc[:],
                                     start=True, stop=True)
                    xT = p2s.tile([128, DT, w], F32R, tag="xt", name=f"xt{blk}")
                    for dt in range(DT):
                        nc.scalar.copy(xT[:, dt, :], pxs[dt][:])
                    rcp = p2st.tile([128, w // 128], F32, tag="rcp",
                                    name=f"rcp{blk}")
                    for j in range(w // 128):
                        dg = p2st.tile([128, 128], F32, tag="dg",
                                       name=f"dg{blk}_{j}")
                        nc.vector.tensor_mul(dg[:], psm[:, j * 128:(j + 1) * 128],
                                             ident[:])
                        nc.vector.reduce_sum(out=rcp[:, j:j + 1], in_=dg[:],
                                             axis=mybir.AxisListType.X)
                    nc.vector.reciprocal_approx_fast(rcp[:], rcp[:])
                    prev_oproj = (q0, w, xT, rcp)
                oproj_ln(*prev_oproj)
    nc.compile()
    return nc


_NC_CACHE = {}


def _get_nc(variant):
    if variant not in _NC_CACHE:
        _NC_CACHE[variant] = (build_kernel_lean() if variant == "lean"
                              else build_kernel_general())
    return _NC_CACHE[variant]


def _is_lean(ln_t_g, ln_t_b, ln_i_g, ln_i_b, bq, bkt, bvt, bki, bvi, bo):
    zeros = [ln_t_b, ln_i_b, bq, bkt, bvt, bki, bvi, bo]
    ones = [ln_t_g, ln_i_g]
    return (all(not np.any(np.asarray(a)) for a in zeros)
            and all(np.all(np.asarray(a) == 1.0) for a in ones))


def _prep_core_inputs(lean, text, img, ln_t_g, ln_t_b, ln_i_g, ln_i_b,
                      Wq, bq, Wkt, bkt, Wvt, bvt, Wki, bki, Wvi, bvi, Wo, bo):
    s = np.float32(D) ** -0.5
    ident = np.eye(128, dtype=np.float32)
    bf = ml_dtypes.bfloat16
    in_maps = []
    for core in range(8):
        b, h = core // 2, core % 2
        m_t = b < 2
        x = np.asarray(text[b] if m_t else img[b - 2], np.float32)
        if h == 1:
            x = np.concatenate([x[TQ:], x[:TQ]], axis=0)
        g = np.asarray(ln_t_g if m_t else ln_i_g, np.float32)
        bb = np.asarray(ln_t_b if m_t else ln_i_b, np.float32)
        Wk, bk = (Wkt, bkt) if m_t else (Wki, bki)
        Wv, bv = (Wvt, bvt) if m_t else (Wvi, bvi)
        Wq_, bq_, Wk, bk, Wv, bv, Wo_, bo_ = [
            np.asarray(a, np.float32) for a in (Wq, bq, Wk, bk, Wv, bv, Wo, bo)]
        if lean:
            # K-proj folded into the query side (scores = A (gq gk^T) A^T)
            # and V+O projections folded together (y = U^T A (Wo Wv)^T)
            gq_f = (Wq_ * g[None, :]).T * s
            gk_f = (Wk * g[None, :]).T
            in_maps.append({
                "x": np.ascontiguousarray(x.astype(bf)),
                "gm": np.ascontiguousarray((gq_f @ gk_f.T).astype(bf)),
                "wvo": np.ascontiguousarray(
                    (Wo_ @ (Wv * g[None, :])).T.astype(bf)),
                "identb": np.eye(128, dtype=bf),
            })
        else:
            in_maps.append({
                "x": np.ascontiguousarray(x),
                "gqt": np.ascontiguousarray((Wq_ * g[None, :]).T * s),
                "gkt": np.ascontiguousarray((Wk * g[None, :]).T),
                "gvt": np.ascontiguousarray((Wv * g[None, :]).T),
                "wot": np.ascontiguousarray(Wo_.T),
                "cq": np.ascontiguousarray((Wq_ @ bb + bq_) * s),
                "ck": np.ascontiguousarray(Wk @ bb + bk),
                "cv": np.ascontiguousarray(Wv @ bb + bv),
                "bo": np.ascontiguousarray(bo_),
                "g2t": np.ascontiguousarray(np.asarray(ln_t_g, np.float32)),
                "b2t": np.ascontiguousarray(np.asarray(ln_t_b, np.float32)),
                "g2i": np.ascontiguousarray(np.asarray(ln_i_g, np.float32)),
                "b2i": np.ascontiguousarray(np.asarray(ln_i_b, np.float32)),
                "ident": ident,
                "onesr": np.ones((128, 128), np.float32),
            })
    return in_maps


def kernel(**inputs):
    kr = kernel_raw(**inputs)
    return kr[0]


def kernel_raw(**inputs):
    """Returns (full_output, BassKernelResults)."""
    import time as _time
    lean = _is_lean(**{k: inputs[k] for k in (
        "ln_t_g", "ln_t_b", "ln_i_g", "ln_i_b",
        "bq", "bkt", "bvt", "bki", "bvi", "bo")})
    nc = _get_nc("lean" if lean else "general")
    in_maps = _prep_core_inputs(lean, **inputs)
    res = None
    last_exc = None
    for attempt in range(6):
        try:
            res = run_bass_kernel_spmd(nc, in_maps, core_ids=list(range(8)))
            break
        except Exception as e:  # transient device wedge self-heals in ~1-3 min
            last_exc = e
            if "UNAVAILABLE" not in str(e) and "INTERNAL" not in str(e):
                raise
            _time.sleep(30)
    if res is None:
        raise last_exc
    out = np.zeros((8, S, D), np.float32)
    for core in range(8):
        b, h = core // 2, core % 2
        o2 = res.results[core]["out2"]
        out[b, h * TQ:(h + 1) * TQ] = o2[0]
        out[4 + b, h * TQ:(h + 1) * TQ] = o2[1]
    return out, res
